# revision 6
# baseline (speedup 1.0000x reference)
"""GRU decoder kernel for Trainium2 (Bass/Tile), SPMD over 8 NeuronCores.

Problem: B=64, H=256, T=2000 GRU recurrence + output projection to 128 dims.
  gi = z @ Wih.T + bih            (precomputed on host: tiny, one-time)
  loop t: gh = h @ Whh.T + bhh; r,zg = sigmoid; n = tanh(i_n + r*h_n)
          h = (1-zg)*n + zg*h
  out = hs @ Wout.T + bout        -> (64, 2000, 128) fp32

Sharding: data-parallel over batch, 8 batch rows per core, weights replicated.

On-chip layout is "gate-major": gate/hidden dims on SBUF partitions, batch on
the free dim.  The recurrent matmul keeps Whh.T tiles as the PE stationary
operand (12 tiles of 128x128, bf16 so FWL halves LDWEIGHTS).  The h operand is
fed split-precision (h = h_hi + h_lo, both bf16, accumulated by two matmuls
per weight tile) so matmul precision is ~fp32 while weight loads stay bf16.
The constant i-gates + bhh bias is injected directly into PSUM with an
identity-rhs matmul so the sigmoid reads PSUM directly.  Hidden states are
stored fp32 in SBUF (128KB/partition ring over all 2000 steps) and consumed
in place as the projection's stationary operand.
"""

import sys

sys.path.insert(0, "/opt/trn_rl_repo")

import numpy as np
import ml_dtypes
from contextlib import ExitStack

import concourse.bass as bass
import concourse.tile as tile
from concourse import bacc, mybir
from concourse import bass_utils

F32 = mybir.dt.float32
BF16 = mybir.dt.bfloat16
AF = mybir.ActivationFunctionType

H = 256
B = 64
NCORES = 8
BL = B // NCORES  # 8 batch rows per core
OUT_D = 128
PROJ_CHUNK = 16  # timesteps per projection matmul (16*8 batch = 128 = M)


def build_program(T, debug=False, enable_asserts=False):
    """Build + compile the per-core Bass program (same program on all cores)."""
    nc = bacc.Bacc(
        "TRN2",
        debug=debug,
        enable_asserts=enable_asserts,
        target_bir_lowering=False,
        num_devices=NCORES,
    )

    SL = 2 * BL  # 16 columns per h slot: [kc0 b0..7 | kc1 b0..7]

    # DRAM inputs (already in final on-chip (partition, free) layout, host-prepped)
    w_dram = nc.dram_tensor("w_tiles", (128, 12 * 128), BF16, kind="ExternalInput")
    crz_dram = nc.dram_tensor("crz_stat", (4 * BL, 128), BF16, kind="ExternalInput")
    cn_dram = nc.dram_tensor("cn_stat", (2 * BL, 128), BF16, kind="ExternalInput")
    i32_dram = nc.dram_tensor("ident32", (4 * BL, 4 * BL), BF16, kind="ExternalInput")
    i16_dram = nc.dram_tensor("ident16", (2 * BL, 2 * BL), BF16, kind="ExternalInput")
    cin_dram = nc.dram_tensor("cin_n", (128, SL), F32, kind="ExternalInput")
    wout_dram = nc.dram_tensor("wout_t", (128, 2 * OUT_D), F32, kind="ExternalInput")
    ones_dram = nc.dram_tensor("ones1", (1, OUT_D), F32, kind="ExternalInput")
    bout_dram = nc.dram_tensor("bout_row", (1, OUT_D), F32, kind="ExternalInput")
    out_dram = nc.dram_tensor("out", (BL, T, OUT_D), F32, kind="ExternalOutput")

    with tile.TileContext(nc) as tc, ExitStack() as ctx:
        const = ctx.enter_context(tc.tile_pool(name="const", bufs=1))
        hsbuf = ctx.enter_context(tc.tile_pool(name="hsbuf", bufs=1))
        work = ctx.enter_context(tc.tile_pool(name="work", bufs=3))
        prz_pool = ctx.enter_context(tc.tile_pool(name="przp", bufs=2, space="PSUM"))
        pn_pool = ctx.enter_context(tc.tile_pool(name="pnp", bufs=2, space="PSUM"))
        pout_pool = ctx.enter_context(tc.tile_pool(name="poutp", bufs=2, space="PSUM"))

        wsb = const.tile([128, 12 * 128], BF16)
        crz = const.tile([4 * BL, 128], BF16)
        cns = const.tile([2 * BL, 128], BF16)
        i32 = const.tile([4 * BL, 4 * BL], BF16)
        i16 = const.tile([2 * BL, 2 * BL], BF16)
        cin = const.tile([128, SL], F32)
        wout = const.tile([128, 2 * OUT_D], F32)
        ones1 = const.tile([1, OUT_D], F32)
        boutr = const.tile([1, OUT_D], F32)

        nc.sync.dma_start(wsb[:], w_dram[:])
        nc.sync.dma_start(crz[:], crz_dram[:])
        nc.sync.dma_start(cns[:], cn_dram[:])
        nc.sync.dma_start(i32[:], i32_dram[:])
        nc.sync.dma_start(i16[:], i16_dram[:])
        nc.sync.dma_start(cin[:], cin_dram[:])
        nc.sync.dma_start(wout[:], wout_dram[:])
        nc.sync.dma_start(ones1[:], ones_dram[:])
        nc.sync.dma_start(boutr[:], bout_dram[:])

        # fp32 hidden-state ring: slot s holds h after step s-1 (slot 0 = zeros)
        hs = hsbuf.tile([128, (T + 1) * SL], F32)
        nc.vector.memset(hs[:, 0:SL], 0.0)

        # initial bf16 hi/lo split of h (zeros)
        hhi = work.tile([128, SL], BF16, tag="hhi")
        hlo = work.tile([128, SL], BF16, tag="hlo")
        nc.vector.memset(hhi[:], 0.0)
        nc.vector.memset(hlo[:], 0.0)

        def wtile(kc, mc):
            return wsb[:, (kc * 6 + mc) * 128 : (kc * 6 + mc + 1) * 128]

        for t in range(T):
            hin = hs[:, t * SL : (t + 1) * SL]
            hout = hs[:, (t + 1) * SL : (t + 2) * SL]

            prz = prz_pool.tile([128, 4 * BL], F32)
            pn = pn_pool.tile([128, 2 * BL], F32)

            # bias seeds: prz = i_rz + bhh_rz (per batch), pn = bhh_n
            nc.tensor.matmul(prz[:], crz[:], i32[:], start=True, stop=True)
            for mc in range(4):  # r0 r1 z0 z1
                for kc in range(2):
                    for hp in (hhi, hlo):
                        nc.tensor.matmul(
                            prz[:, mc * BL : (mc + 1) * BL],
                            wtile(kc, mc),
                            hp[:, kc * BL : (kc + 1) * BL],
                            start=False,
                            stop=(mc == 3 and kc == 1 and hp is hlo),
                            skip_group_check=True,
                        )
            nc.tensor.matmul(pn[:], cns[:], i16[:], start=True, stop=True)
            for mc in range(4, 6):  # n0 n1
                for kc in range(2):
                    for hp in (hhi, hlo):
                        nc.tensor.matmul(
                            pn[:, (mc - 4) * BL : (mc - 3) * BL],
                            wtile(kc, mc),
                            hp[:, kc * BL : (kc + 1) * BL],
                            start=False,
                            stop=(mc == 5 and kc == 1 and hp is hlo),
                            skip_group_check=True,
                        )

            srz = work.tile([128, 4 * BL], F32, tag="srz")
            zq = work.tile([128, 2 * BL], F32, tag="zq")
            t1 = work.tile([128, 2 * BL], F32, tag="t1")
            t2 = work.tile([128, 2 * BL], F32, tag="t2")
            nt = work.tile([128, 2 * BL], F32, tag="nt")
            zh = work.tile([128, 2 * BL], F32, tag="zh")
            m = work.tile([128, 2 * BL], F32, tag="m")

            # r | z = sigmoid(prz); zq = 1 - z = sigmoid(-pre_z)
            nc.scalar.activation(srz[:], prz[:], AF.Sigmoid)
            nc.scalar.activation(zq[:], prz[:, 2 * BL : 4 * BL], AF.Sigmoid, scale=-1.0)
            # n = tanh(i_n + r * (gh_n + bhh_n))
            nc.vector.tensor_mul(t1[:], srz[:, 0 : 2 * BL], pn[:])
            nc.vector.tensor_add(t2[:], t1[:], cin[:])
            nc.scalar.activation(nt[:], t2[:], AF.Tanh)
            # h' = (1-z)*n + z*h
            nc.vector.tensor_mul(zh[:], srz[:, 2 * BL : 4 * BL], hin[:])
            nc.vector.tensor_mul(m[:], zq[:], nt[:])
            nc.vector.tensor_add(hout[:], m[:], zh[:])
            # split h' into bf16 hi + lo for the next matmul
            hhi = work.tile([128, SL], BF16, tag="hhi")
            hlo = work.tile([128, SL], BF16, tag="hlo")
            nc.vector.tensor_copy(hhi[:], hout[:])
            nc.vector.tensor_sub(hlo[:], hout[:], hhi[:])

        # ---- projection: out[b, t, :] = hs[b, t] @ Wout.T + bout ----
        hs3 = hs[:].rearrange("p (s c) -> p s c", c=SL)
        t0 = 0
        while t0 < T:
            csz = min(PROJ_CHUNK, T - t0)
            mm = csz * BL
            ps = pout_pool.tile([mm, OUT_D], F32, tag="ps")
            nc.tensor.matmul(ps[:], ones1[:, 0:mm], boutr[:], start=True, stop=True)
            for kc in range(2):
                stg = work.tile([128, mm], F32, tag=f"stgl{kc}")
                nc.vector.tensor_copy(
                    stg[:], hs3[:, t0 + 1 : t0 + 1 + csz, kc * BL : (kc + 1) * BL]
                )
                nc.tensor.matmul(
                    ps[:],
                    stg[:],
                    wout[:, kc * OUT_D : (kc + 1) * OUT_D],
                    start=False,
                    stop=(kc == 1),
                    skip_group_check=True,
                )
            stage = work.tile([mm, OUT_D], F32, tag="stage")
            nc.scalar.copy(stage[:], ps[:])
            dst = out_dram.rearrange("b t d -> t b d")[t0 : t0 + csz, :, :]
            nc.sync.dma_start(dst, stage[:])
            t0 += csz

    nc.compile()
    return nc


def host_prep(z, Wih, bih, Whh, bhh, Wout, bout, T):
    """Numpy preprocessing into per-core on-chip layouts."""
    z = np.asarray(z, np.float32)
    gi = z @ np.asarray(Wih, np.float32).T + np.asarray(bih, np.float32)  # (B, 768)
    bhh = np.asarray(bhh, np.float32)
    WhhT = np.ascontiguousarray(np.asarray(Whh, np.float32).T)  # (256, 768)
    # stationary weight tiles: wsb[k, (kc*6+mc)*128+j] = WhhT[kc*128+k, mc*128+j]
    wsb = (
        WhhT.reshape(2, 128, 6, 128)
        .transpose(1, 0, 2, 3)
        .reshape(128, 12 * 128)
        .astype(ml_dtypes.bfloat16)
    )
    WoutT = np.asarray(Wout, np.float32).T  # (256, 128)
    wout_t = np.ascontiguousarray(
        WoutT.reshape(2, 128, OUT_D).transpose(1, 0, 2).reshape(128, 2 * OUT_D)
    ).astype(np.float32)
    i32 = np.eye(4 * BL, dtype=ml_dtypes.bfloat16)
    i16 = np.eye(2 * BL, dtype=ml_dtypes.bfloat16)
    ones1 = np.ones((1, OUT_D), np.float32)
    bout_row = np.asarray(bout, np.float32).reshape(1, OUT_D)
    cn_stat = (
        np.repeat(bhh[512:].reshape(2, 1, 128), BL, axis=1)
        .reshape(2 * BL, 128)
        .astype(ml_dtypes.bfloat16)
    )

    in_maps = []
    for c in range(NCORES):
        gic = gi[c * BL : (c + 1) * BL]  # (BL, 768)
        Crz = gic[:, :512] + bhh[:512]  # (BL, 512)
        crz_stat = (
            Crz.reshape(BL, 4, 128).transpose(1, 0, 2).reshape(4 * BL, 128)
        ).astype(ml_dtypes.bfloat16)
        cin = np.ascontiguousarray(
            gic[:, 512:].reshape(BL, 2, 128).transpose(2, 1, 0).reshape(128, 2 * BL)
        ).astype(np.float32)
        in_maps.append(
            {
                "w_tiles": wsb,
                "crz_stat": crz_stat,
                "cn_stat": cn_stat,
                "ident32": i32,
                "ident16": i16,
                "cin_n": cin,
                "wout_t": wout_t,
                "ones1": ones1,
                "bout_row": bout_row,
            }
        )
    return in_maps


_CACHED = {}


def _get_program(T):
    if T not in _CACHED:
        _CACHED[T] = build_program(T)
    return _CACHED[T]


def run(z, Wih, bih, Whh, bhh, Wout, bout, n_frames, trace=False):
    T = int(n_frames)
    nc = _get_program(T)
    in_maps = host_prep(z, Wih, bih, Whh, bhh, Wout, bout, T)
    res = bass_utils.run_bass_kernel_spmd(
        nc, in_maps, core_ids=list(range(NCORES)), trace=trace
    )
    out = np.concatenate([res.results[c]["out"] for c in range(NCORES)], axis=0)
    return out.astype(np.float32), res


def kernel(z, Wih, bih, Whh, bhh, Wout, bout, n_frames):
    out, _ = run(z, Wih, bih, Whh, bhh, Wout, bout, n_frames)
    return out


# revision 7
# speedup vs baseline: 1.4980x; 1.4980x over previous
"""GRU decoder kernel for Trainium2 (Bass/Tile), SPMD over 8 NeuronCores.

Problem: B=64, H=256, T=2000 GRU recurrence + output projection to 128 dims.
  gi = z @ Wih.T + bih            (precomputed on host: tiny, one-time)
  loop t: gh = h @ Whh.T + bhh; r,zg = sigmoid; n = tanh(i_n + r*h_n)
          h = (1-zg)*n + zg*h
  out = hs @ Wout.T + bout        -> (64, 2000, 128) fp32

Sharding: data-parallel over batch, 8 batch rows per core, weights replicated.

Layout is "gate-major": gate/hidden dims on SBUF partitions, batch on the free
dim.  The recurrent matmul keeps Whh.T tiles as the PE stationary operand
(12 tiles of 128x128, bf16 so FWL halves LDWEIGHTS); the moving operand is a
bf16 cast of h (the fp32 master state is carried in SBUF, so only the damped
matmul path sees bf16 — measured end-to-end error ~1e-3).  The constant
i-gates + bhh bias is injected directly into PSUM with an identity-rhs matmul
so the sigmoid reads PSUM directly.  The three gates use separate PSUM banks
ordered r, n, z so the r-sigmoid and the tanh path start before the sweep
finishes.  Hidden states are stored fp32 in SBUF (128KB/partition ring over
all 2000 steps) and consumed in place as the projection's stationary operand.
"""

import sys

sys.path.insert(0, "/opt/trn_rl_repo")

import numpy as np
import ml_dtypes
from contextlib import ExitStack

import concourse.bass as bass
import concourse.tile as tile
from concourse import bacc, mybir
from concourse import bass_utils

F32 = mybir.dt.float32
BF16 = mybir.dt.bfloat16
AF = mybir.ActivationFunctionType

H = 256
B = 64
NCORES = 8
BL = B // NCORES  # 8 batch rows per core
OUT_D = 128
PROJ_CHUNK = 16  # timesteps per projection matmul (16*8 batch = 128 = M)

# gate order within the sweep: r first (feeds sigmoid early), n second
# (feeds the tanh chain), z last (its consumers run during the tanh)
GATE_MC = {"r": (0, 1), "z": (2, 3), "n": (4, 5)}


def build_program(T, debug=False, enable_asserts=False):
    """Build + compile the per-core Bass program (same program on all cores)."""
    nc = bacc.Bacc(
        "TRN2",
        debug=debug,
        enable_asserts=enable_asserts,
        target_bir_lowering=False,
        num_devices=NCORES,
    )

    SL = 2 * BL  # 16 columns per h slot: [kc0 b0..7 | kc1 b0..7]

    # DRAM inputs (already in final on-chip (partition, free) layout, host-prepped)
    w_dram = nc.dram_tensor("w_tiles", (128, 12 * 128), BF16, kind="ExternalInput")
    cr_dram = nc.dram_tensor("cr_stat", (SL, 128), BF16, kind="ExternalInput")
    cz_dram = nc.dram_tensor("cz_stat", (SL, 128), BF16, kind="ExternalInput")
    cn_dram = nc.dram_tensor("cn_stat", (SL, 128), BF16, kind="ExternalInput")
    i16_dram = nc.dram_tensor("ident16", (SL, SL), BF16, kind="ExternalInput")
    cin_dram = nc.dram_tensor("cin_n", (128, SL), F32, kind="ExternalInput")
    wout_dram = nc.dram_tensor("wout_t", (128, 2 * OUT_D), F32, kind="ExternalInput")
    ones_dram = nc.dram_tensor("ones1", (1, OUT_D), F32, kind="ExternalInput")
    bout_dram = nc.dram_tensor("bout_row", (1, OUT_D), F32, kind="ExternalInput")
    out_dram = nc.dram_tensor("out", (BL, T, OUT_D), F32, kind="ExternalOutput")

    with tile.TileContext(nc) as tc, ExitStack() as ctx:
        const = ctx.enter_context(tc.tile_pool(name="const", bufs=1))
        hsbuf = ctx.enter_context(tc.tile_pool(name="hsbuf", bufs=1))
        work = ctx.enter_context(tc.tile_pool(name="work", bufs=3))
        pr_pool = ctx.enter_context(tc.tile_pool(name="prp", bufs=2, space="PSUM"))
        pn_pool = ctx.enter_context(tc.tile_pool(name="pnp", bufs=2, space="PSUM"))
        pz_pool = ctx.enter_context(tc.tile_pool(name="pzp", bufs=2, space="PSUM"))
        pout_pool = ctx.enter_context(tc.tile_pool(name="poutp", bufs=2, space="PSUM"))

        wsb = const.tile([128, 12 * 128], BF16)
        crs = const.tile([SL, 128], BF16)
        czs = const.tile([SL, 128], BF16)
        cns = const.tile([SL, 128], BF16)
        i16 = const.tile([SL, SL], BF16)
        cin = const.tile([128, SL], F32)
        wout = const.tile([128, 2 * OUT_D], F32)
        ones1 = const.tile([1, OUT_D], F32)
        boutr = const.tile([1, OUT_D], F32)

        nc.sync.dma_start(wsb[:], w_dram[:])
        nc.sync.dma_start(crs[:], cr_dram[:])
        nc.sync.dma_start(czs[:], cz_dram[:])
        nc.sync.dma_start(cns[:], cn_dram[:])
        nc.sync.dma_start(i16[:], i16_dram[:])
        nc.sync.dma_start(cin[:], cin_dram[:])
        nc.sync.dma_start(wout[:], wout_dram[:])
        nc.sync.dma_start(ones1[:], ones_dram[:])
        nc.sync.dma_start(boutr[:], bout_dram[:])

        # fp32 hidden-state ring: slot s holds h after step s-1 (slot 0 = zeros)
        hs = hsbuf.tile([128, (T + 1) * SL], F32)
        nc.vector.memset(hs[:, 0:SL], 0.0)

        # bf16 cast of h for the matmul moving operand
        hbf = work.tile([128, SL], BF16, tag="hbf")
        nc.vector.memset(hbf[:], 0.0)

        def wtile(kc, mc):
            return wsb[:, (kc * 6 + mc) * 128 : (kc * 6 + mc + 1) * 128]

        def gate_sweep(psum, cstat, gate, hbf):
            nc.tensor.matmul(psum[:], cstat[:], i16[:], start=True, stop=True)
            mcs = GATE_MC[gate]
            for i, mc in enumerate(mcs):
                for kc in range(2):
                    nc.tensor.matmul(
                        psum[:, i * BL : (i + 1) * BL],
                        wtile(kc, mc),
                        hbf[:, kc * BL : (kc + 1) * BL],
                        start=False,
                        stop=(i == 1 and kc == 1),
                        skip_group_check=True,
                    )

        for t in range(T):
            hin = hs[:, t * SL : (t + 1) * SL]
            hout = hs[:, (t + 1) * SL : (t + 2) * SL]

            pr = pr_pool.tile([128, SL], F32)
            pn = pn_pool.tile([128, SL], F32)
            pz = pz_pool.tile([128, SL], F32)

            gate_sweep(pr, crs, "r", hbf)
            gate_sweep(pn, cns, "n", hbf)
            gate_sweep(pz, czs, "z", hbf)

            sr = work.tile([128, SL], F32, tag="sr")
            sz = work.tile([128, SL], F32, tag="sz")
            zq = work.tile([128, SL], F32, tag="zq")
            t1 = work.tile([128, SL], F32, tag="t1")
            t2 = work.tile([128, SL], F32, tag="t2")
            nt = work.tile([128, SL], F32, tag="nt")
            zh = work.tile([128, SL], F32, tag="zh")
            m = work.tile([128, SL], F32, tag="m")

            # ACT queue order: sigmoid(r), sigmoid(z), 1-sigmoid(z), tanh
            nc.scalar.activation(sr[:], pr[:], AF.Sigmoid)
            nc.scalar.activation(sz[:], pz[:], AF.Sigmoid)
            nc.scalar.activation(zq[:], pz[:], AF.Sigmoid, scale=-1.0)
            # n = tanh(i_n + r * (gh_n + bhh_n))
            nc.vector.tensor_mul(t1[:], sr[:], pn[:])
            nc.vector.tensor_add(t2[:], t1[:], cin[:])
            nc.scalar.activation(nt[:], t2[:], AF.Tanh)
            # h' = (1-z)*n + z*h
            nc.vector.tensor_mul(zh[:], sz[:], hin[:])
            nc.vector.tensor_mul(m[:], zq[:], nt[:])
            nc.vector.tensor_add(hout[:], m[:], zh[:])
            hbf = work.tile([128, SL], BF16, tag="hbf")
            nc.vector.tensor_copy(hbf[:], hout[:])

        # ---- projection: out[b, t, :] = hs[b, t] @ Wout.T + bout ----
        hs3 = hs[:].rearrange("p (s c) -> p s c", c=SL)
        t0 = 0
        while t0 < T:
            csz = min(PROJ_CHUNK, T - t0)
            mm = csz * BL
            ps = pout_pool.tile([mm, OUT_D], F32, tag="ps")
            nc.tensor.matmul(ps[:], ones1[:, 0:mm], boutr[:], start=True, stop=True)
            for kc in range(2):
                stg = work.tile([128, mm], F32, tag=f"stgl{kc}")
                nc.vector.tensor_copy(
                    stg[:], hs3[:, t0 + 1 : t0 + 1 + csz, kc * BL : (kc + 1) * BL]
                )
                nc.tensor.matmul(
                    ps[:],
                    stg[:],
                    wout[:, kc * OUT_D : (kc + 1) * OUT_D],
                    start=False,
                    stop=(kc == 1),
                    skip_group_check=True,
                )
            stage = work.tile([mm, OUT_D], F32, tag="stage")
            nc.scalar.copy(stage[:], ps[:])
            dst = out_dram.rearrange("b t d -> t b d")[t0 : t0 + csz, :, :]
            nc.sync.dma_start(dst, stage[:])
            t0 += csz

    nc.compile()
    return nc


def host_prep(z, Wih, bih, Whh, bhh, Wout, bout, T):
    """Numpy preprocessing into per-core on-chip layouts."""
    z = np.asarray(z, np.float32)
    gi = z @ np.asarray(Wih, np.float32).T + np.asarray(bih, np.float32)  # (B, 768)
    bhh = np.asarray(bhh, np.float32)
    WhhT = np.ascontiguousarray(np.asarray(Whh, np.float32).T)  # (256, 768)
    # stationary weight tiles: wsb[k, (kc*6+mc)*128+j] = WhhT[kc*128+k, mc*128+j]
    wsb = (
        WhhT.reshape(2, 128, 6, 128)
        .transpose(1, 0, 2, 3)
        .reshape(128, 12 * 128)
        .astype(ml_dtypes.bfloat16)
    )
    WoutT = np.asarray(Wout, np.float32).T  # (256, 128)
    wout_t = np.ascontiguousarray(
        WoutT.reshape(2, 128, OUT_D).transpose(1, 0, 2).reshape(128, 2 * OUT_D)
    ).astype(np.float32)
    i16 = np.eye(2 * BL, dtype=ml_dtypes.bfloat16)
    ones1 = np.ones((1, OUT_D), np.float32)
    bout_row = np.asarray(bout, np.float32).reshape(1, OUT_D)
    cn_stat = (
        np.repeat(bhh[512:].reshape(2, 1, 128), BL, axis=1)
        .reshape(2 * BL, 128)
        .astype(ml_dtypes.bfloat16)
    )

    in_maps = []
    for c in range(NCORES):
        gic = gi[c * BL : (c + 1) * BL]  # (BL, 768)
        Crz = gic[:, :512] + bhh[:512]  # (BL, 512)
        crz_stat = Crz.reshape(BL, 4, 128).transpose(1, 0, 2).reshape(4 * BL, 128)
        cr_stat = crz_stat[0 : 2 * BL].astype(ml_dtypes.bfloat16)
        cz_stat = crz_stat[2 * BL : 4 * BL].astype(ml_dtypes.bfloat16)
        cin = np.ascontiguousarray(
            gic[:, 512:].reshape(BL, 2, 128).transpose(2, 1, 0).reshape(128, 2 * BL)
        ).astype(np.float32)
        in_maps.append(
            {
                "w_tiles": wsb,
                "cr_stat": cr_stat,
                "cz_stat": cz_stat,
                "cn_stat": cn_stat,
                "ident16": i16,
                "cin_n": cin,
                "wout_t": wout_t,
                "ones1": ones1,
                "bout_row": bout_row,
            }
        )
    return in_maps


_CACHED = {}


def _get_program(T):
    if T not in _CACHED:
        _CACHED[T] = build_program(T)
    return _CACHED[T]


def run(z, Wih, bih, Whh, bhh, Wout, bout, n_frames, trace=False):
    T = int(n_frames)
    nc = _get_program(T)
    in_maps = host_prep(z, Wih, bih, Whh, bhh, Wout, bout, T)
    res = bass_utils.run_bass_kernel_spmd(
        nc, in_maps, core_ids=list(range(NCORES)), trace=trace
    )
    out = np.concatenate([res.results[c]["out"] for c in range(NCORES)], axis=0)
    return out.astype(np.float32), res


def kernel(z, Wih, bih, Whh, bhh, Wout, bout, n_frames):
    out, _ = run(z, Wih, bih, Whh, bhh, Wout, bout, n_frames)
    return out
